# revision 2
# baseline (speedup 1.0000x reference)
"""Bass/Trainium2 kernel for nn_GPT2FFNInputModel (segment_reduce, memory regime).

Reference computes, for B=16 gathered token rows x[b] = ffn_input[b, pos[b]]:
    out[b] = mean_f( x[b] @ W[tl] + b[tl] )        (masked to 0 for invalid pos)

The mean over F folds through the matmul:
    out[b] = (x[b] . w_sum) / F + mean(b[tl]),   w_sum[d] = sum_f W[tl][d, f]

so the only bulk memory work is the row-sum (segment reduce) of W[tl]
(768 x 3072 f32 = 9.4 MB).  That reduction runs on 8 NeuronCores, each
reducing a contiguous 1/8th of W[tl] laid out as [128 partitions x 2304],
producing 768-element block partial sums.  The tiny [16,768] gather, the
16x768 dot, bias mean and validity mask run on host (48 KB of data).
"""

import numpy as np

import concourse.bass as bass
import concourse.mybir as mybir
import concourse.tile as tile
from concourse import bacc
from concourse.bass_utils import run_bass_kernel_spmd

B, S, D, F = 16, 2048, 768, 3072
N_CORES = 8
P = 128
ELEMS_PER_CORE = D * F // N_CORES      # 294912 contiguous f32 per core
COLS = ELEMS_PER_CORE // P             # 2304 per partition
BLK = 768                              # reduction block; F % BLK == 0 keeps
NBLK = COLS // BLK                     # 3   row boundaries block-aligned

_NC_CACHE = {}


def _build_nc(n_dma=NBLK):
    """One core's program: DMA [128, 2304] f32 in `n_dma` column tiles,
    VectorE-reduce each tile over its free dim in BLK-sized chunks,
    DMA the [128, NBLK] block sums out."""
    nc = bacc.Bacc(None, target_bir_lowering=False)
    w = nc.declare_dram_parameter("w", [P, COLS], mybir.dt.float32, isOutput=False)
    out = nc.declare_dram_parameter("out", [P, NBLK], mybir.dt.float32, isOutput=True)

    tile_cols = COLS // n_dma
    blk_per_tile = tile_cols // BLK

    with tile.TileContext(nc) as tc:
        with (
            tc.tile_pool(name="wpool", bufs=min(3, n_dma)) as wp,
            tc.tile_pool(name="opool", bufs=1) as op,
        ):
            ot = op.tile([P, NBLK], mybir.dt.float32)
            for j in range(n_dma):
                t = wp.tile([P, tile_cols], mybir.dt.float32)
                nc.sync.dma_start(out=t[:], in_=w[:, j * tile_cols:(j + 1) * tile_cols])
                if blk_per_tile == 1:
                    nc.vector.tensor_reduce(
                        out=ot[:, j:j + 1], in_=t[:],
                        axis=mybir.AxisListType.X, op=mybir.AluOpType.add,
                    )
                else:
                    nc.vector.tensor_reduce(
                        out=ot[:, j * blk_per_tile:(j + 1) * blk_per_tile],
                        in_=t[:].rearrange("p (g d) -> p g d", g=blk_per_tile),
                        axis=mybir.AxisListType.X, op=mybir.AluOpType.add,
                    )
            nc.sync.dma_start(out=out[:], in_=ot[:])
    nc.compile()
    return nc


def _get_nc(n_dma=NBLK):
    if n_dma not in _NC_CACHE:
        _NC_CACHE[n_dma] = _build_nc(n_dma)
    return _NC_CACHE[n_dma]


def _run_device(wl_flat, n_dma=NBLK, trace=False):
    """wl_flat: contiguous f32 [D*F]. Returns (w_sum [D] f64, results obj)."""
    in_maps = [
        {"w": np.ascontiguousarray(
            wl_flat[c * ELEMS_PER_CORE:(c + 1) * ELEMS_PER_CORE].reshape(P, COLS))}
        for c in range(N_CORES)
    ]
    res = run_bass_kernel_spmd(
        _get_nc(n_dma), in_maps, core_ids=list(range(N_CORES)), trace=trace
    )
    blocks = np.concatenate(
        [np.asarray(r["out"], dtype=np.float64).reshape(-1) for r in res.results]
    )                                   # 3072 sums of 768 consecutive flat elems
    w_sum = blocks.reshape(D, F // BLK).sum(axis=1)     # [768]
    return w_sum, res


def kernel(ffn_input, W, b, target_layer, target_token_positions):
    tl = int(target_layer)
    wl_flat = np.ascontiguousarray(W[tl], dtype=np.float32).reshape(-1)
    w_sum, _ = _run_device(wl_flat)

    pos = np.asarray(target_token_positions).astype(np.int64)
    valid = (pos >= 0) & (pos < S)
    safe = np.clip(pos, 0, S - 1)
    x = np.asarray(ffn_input)[np.arange(B), safe].astype(np.float64)   # [16, 768]
    row = x @ w_sum / F + float(np.asarray(b[tl], dtype=np.float64).mean())
    return np.where(valid, row, 0.0).astype(np.float32)


# revision 7
# speedup vs baseline: 1.0881x; 1.0881x over previous
"""Bass/Trainium2 kernel for nn_GPT2FFNInputModel (segment_reduce, memory regime).

Reference computes, for B=16 gathered token rows x[b] = ffn_input[b, pos[b]]:
    out[b] = mean_f( x[b] @ W[tl] + b[tl] )        (masked to 0 for invalid pos)

The mean over F folds through the matmul:
    out[b] = (x[b] . w_sum) / F + mean(b[tl]),   w_sum[d] = sum_f W[tl][d, f]

so the only bulk memory work is the row-sum (segment reduce) of W[tl]
(768 x 3072 f32 = 9.4 MB).  That reduction runs on 8 NeuronCores, each
reducing a contiguous 1/8th of W[tl] laid out as [128 partitions x 2304],
producing 768-element block partial sums.  The tiny [16,768] gather, the
16x768 dot, bias mean and validity mask run on host (48 KB of data).
"""

from contextlib import ExitStack

import numpy as np

import concourse.bass as bass
import concourse.mybir as mybir
import concourse.tile as tile
from concourse import bacc
from concourse.bass_utils import run_bass_kernel_spmd

B, S, D, F = 16, 2048, 768, 3072
N_CORES = 8
P = 128
ELEMS_PER_CORE = D * F // N_CORES      # 294912 contiguous f32 per core
COLS = ELEMS_PER_CORE // P             # 2304 per partition
BLK = 768                              # reduction block; F % BLK == 0 keeps
NBLK = COLS // BLK                     # 3   row boundaries block-aligned

VARIANT = "raw4"                       # which device program kernel() uses

_NC_CACHE = {}


def _build_nc_raw(n_tiles=4):
    """Raw bass (no TileContext): explicit semaphores, minimal engine set.
    Sync and Scalar (both HWDGE) each issue half the input DMAs in
    parallel; VectorE reduces each tile as it lands; Sync DMAs the block
    sums out.  Avoids Tile's multi-microsecond entry/exit barriers."""
    tile_cols = COLS // n_tiles                  # per-tile free dim
    blk = 768
    while tile_cols % blk:                       # largest BLK dividing both
        blk //= 2                                # tile_cols and F
    g = tile_cols // blk
    nblk_total = COLS // blk

    nc = bass.Bass(target_bir_lowering=False)
    w = nc.declare_dram_parameter("w", [P, COLS], mybir.dt.float32, isOutput=False)
    out = nc.declare_dram_parameter(
        "out", [P, nblk_total], mybir.dt.float32, isOutput=True
    )

    with ExitStack() as ctx:
        s_sem = ctx.enter_context(nc.semaphore("s_sem"))
        a_sem = ctx.enter_context(nc.semaphore("a_sem"))
        v_sem = ctx.enter_context(nc.semaphore("v_sem"))
        tiles = [
            ctx.enter_context(
                nc.sbuf_tensor(f"t{j}", [P, tile_cols], mybir.dt.float32)
            )
            for j in range(n_tiles)
        ]
        ot = ctx.enter_context(
            nc.sbuf_tensor("ot", [P, nblk_total], mybir.dt.float32)
        )

        # tile j -> (engine, completion threshold on that engine's sem)
        half = (n_tiles + 1) // 2
        owner = [("s", 16 * (j + 1)) if j < half else ("a", 16 * (j - half + 1))
                 for j in range(n_tiles)]

        with nc.Block() as block:

            @block.sync
            def _(sync):
                for j in range(n_tiles):
                    if owner[j][0] == "s":
                        sync.dma_start(
                            out=tiles[j][:],
                            in_=w[:, j * tile_cols:(j + 1) * tile_cols],
                        ).then_inc(s_sem, 16)
                sync.wait_ge(v_sem, n_tiles)
                sync.dma_start(out=out[:], in_=ot[:]).then_inc(s_sem, 16)
                sync.wait_ge(s_sem, 16 * (half + 1))

            @block.scalar
            def _(scalar):
                for j in range(n_tiles):
                    if owner[j][0] == "a":
                        scalar.dma_start(
                            out=tiles[j][:],
                            in_=w[:, j * tile_cols:(j + 1) * tile_cols],
                        ).then_inc(a_sem, 16)

            @block.vector
            def _(vector):
                # chase the two DMA streams in arrival order
                order = sorted(range(n_tiles), key=lambda j: (owner[j][1], j))
                for j in order:
                    sem = s_sem if owner[j][0] == "s" else a_sem
                    vector.wait_ge(sem, owner[j][1])
                    if g == 1:
                        src = tiles[j][:]
                    else:
                        src = tiles[j][:].rearrange("p (g d) -> p g d", g=g)
                    vector.tensor_reduce(
                        out=ot[:, j * g:(j + 1) * g],
                        in_=src,
                        axis=mybir.AxisListType.X,
                        op=mybir.AluOpType.add,
                    ).then_inc(v_sem, 1)

    return nc, blk


def _build_nc(n_dma=NBLK):
    """One core's program: DMA [128, 2304] f32 in `n_dma` column tiles,
    VectorE-reduce each tile over its free dim in BLK-sized chunks,
    DMA the [128, NBLK] block sums out."""
    nc = bacc.Bacc(None, target_bir_lowering=False)
    w = nc.declare_dram_parameter("w", [P, COLS], mybir.dt.float32, isOutput=False)
    out = nc.declare_dram_parameter("out", [P, NBLK], mybir.dt.float32, isOutput=True)

    tile_cols = COLS // n_dma
    blk_per_tile = tile_cols // BLK

    with tile.TileContext(nc) as tc:
        with (
            tc.tile_pool(name="wpool", bufs=min(3, n_dma)) as wp,
            tc.tile_pool(name="opool", bufs=1) as op,
        ):
            ot = op.tile([P, NBLK], mybir.dt.float32)
            for j in range(n_dma):
                t = wp.tile([P, tile_cols], mybir.dt.float32)
                nc.sync.dma_start(out=t[:], in_=w[:, j * tile_cols:(j + 1) * tile_cols])
                if blk_per_tile == 1:
                    nc.vector.tensor_reduce(
                        out=ot[:, j:j + 1], in_=t[:],
                        axis=mybir.AxisListType.X, op=mybir.AluOpType.add,
                    )
                else:
                    nc.vector.tensor_reduce(
                        out=ot[:, j * blk_per_tile:(j + 1) * blk_per_tile],
                        in_=t[:].rearrange("p (g d) -> p g d", g=blk_per_tile),
                        axis=mybir.AxisListType.X, op=mybir.AluOpType.add,
                    )
            nc.sync.dma_start(out=out[:], in_=ot[:])
    nc.compile()
    return nc, BLK


def _get_nc(variant="raw4"):
    if variant not in _NC_CACHE:
        if variant == "tile":
            _NC_CACHE[variant] = _build_nc()
        else:
            _NC_CACHE[variant] = _build_nc_raw(n_tiles=int(variant[3:]))
    return _NC_CACHE[variant]


def _run_device(wl_flat, variant="raw4", trace=False):
    """wl_flat: contiguous f32 [D*F]. Returns (w_sum [D] f64, results obj)."""
    nc, blk = _get_nc(variant)
    in_maps = [
        {"w": np.ascontiguousarray(
            wl_flat[c * ELEMS_PER_CORE:(c + 1) * ELEMS_PER_CORE].reshape(P, COLS))}
        for c in range(N_CORES)
    ]
    res = run_bass_kernel_spmd(
        nc, in_maps, core_ids=list(range(N_CORES)), trace=trace
    )
    blocks = np.concatenate(
        [np.asarray(r["out"], dtype=np.float64).reshape(-1) for r in res.results]
    )                                   # sums of blk consecutive flat elems
    w_sum = blocks.reshape(D, F // blk).sum(axis=1)     # [768]
    return w_sum, res


def kernel(ffn_input, W, b, target_layer, target_token_positions):
    tl = int(target_layer)
    wl_flat = np.ascontiguousarray(W[tl], dtype=np.float32).reshape(-1)
    w_sum, _ = _run_device(wl_flat, variant=VARIANT)

    pos = np.asarray(target_token_positions).astype(np.int64)
    valid = (pos >= 0) & (pos < S)
    safe = np.clip(pos, 0, S - 1)
    x = np.asarray(ffn_input)[np.arange(B), safe].astype(np.float64)   # [16, 768]
    row = x @ w_sum / F + float(np.asarray(b[tl], dtype=np.float64).mean())
    return np.where(valid, row, 0.0).astype(np.float32)


# revision 10
# speedup vs baseline: 1.4737x; 1.3544x over previous
"""Bass/Trainium2 kernel for nn_GPT2FFNInputModel (segment_reduce, memory regime).

Reference computes, for B=16 gathered token rows x[b] = ffn_input[b, pos[b]]:
    out[b] = mean_f( x[b] @ W[tl] + b[tl] )        (masked to 0 for invalid pos)

The mean over F folds through the matmul:
    out[b] = (x[b] . w_sum) / F + mean(b[tl]),   w_sum[d] = sum_f W[tl][d, f]

so the only bulk memory work is the row-sum (segment reduce) of W[tl]
(768 x 3072 f32 = 9.4 MB).  That reduction runs on 8 NeuronCores, each
reducing a contiguous 1/8th of W[tl] laid out as [128 partitions x 2304],
producing 768-element block partial sums.  The tiny [16,768] gather, the
16x768 dot, bias mean and validity mask run on host (48 KB of data).
"""

from contextlib import ExitStack

import numpy as np

import concourse.bass as bass
import concourse.mybir as mybir
import concourse.tile as tile
from concourse import bacc
from concourse.bass_utils import run_bass_kernel_spmd

B, S, D, F = 16, 2048, 768, 3072
N_CORES = 8
P = 128
ELEMS_PER_CORE = D * F // N_CORES      # 294912 contiguous f32 per core
COLS = ELEMS_PER_CORE // P             # 2304 per partition
BLK = 768                              # reduction block; F % BLK == 0 keeps
NBLK = COLS // BLK                     # 3   row boundaries block-aligned

VARIANT = "fast"                       # which device program kernel() uses

_NC_CACHE = {}


def _build_nc_raw(n_tiles=4):
    """Raw bass (no TileContext): explicit semaphores, minimal engine set.
    Sync and Scalar (both HWDGE) each issue half the input DMAs in
    parallel; VectorE reduces each tile as it lands; Sync DMAs the block
    sums out.  Avoids Tile's multi-microsecond entry/exit barriers."""
    tile_cols = COLS // n_tiles                  # per-tile free dim
    blk = 768
    while tile_cols % blk:                       # largest BLK dividing both
        blk //= 2                                # tile_cols and F
    g = tile_cols // blk
    nblk_total = COLS // blk

    nc = bass.Bass(target_bir_lowering=False)
    w = nc.declare_dram_parameter("w", [P, COLS], mybir.dt.float32, isOutput=False)
    out = nc.declare_dram_parameter(
        "out", [P, nblk_total], mybir.dt.float32, isOutput=True
    )

    with ExitStack() as ctx:
        s_sem = ctx.enter_context(nc.semaphore("s_sem"))
        a_sem = ctx.enter_context(nc.semaphore("a_sem"))
        v_sem = ctx.enter_context(nc.semaphore("v_sem"))
        tiles = [
            ctx.enter_context(
                nc.sbuf_tensor(f"t{j}", [P, tile_cols], mybir.dt.float32)
            )
            for j in range(n_tiles)
        ]
        ot = ctx.enter_context(
            nc.sbuf_tensor("ot", [P, nblk_total], mybir.dt.float32)
        )

        # tile j -> (engine, completion threshold on that engine's sem)
        half = (n_tiles + 1) // 2
        owner = [("s", 16 * (j + 1)) if j < half else ("a", 16 * (j - half + 1))
                 for j in range(n_tiles)]

        with nc.Block() as block:

            @block.sync
            def _(sync):
                for j in range(n_tiles):
                    if owner[j][0] == "s":
                        sync.dma_start(
                            out=tiles[j][:],
                            in_=w[:, j * tile_cols:(j + 1) * tile_cols],
                        ).then_inc(s_sem, 16)
                sync.wait_ge(v_sem, n_tiles)
                sync.dma_start(out=out[:], in_=ot[:]).then_inc(s_sem, 16)
                sync.wait_ge(s_sem, 16 * (half + 1))

            @block.scalar
            def _(scalar):
                for j in range(n_tiles):
                    if owner[j][0] == "a":
                        scalar.dma_start(
                            out=tiles[j][:],
                            in_=w[:, j * tile_cols:(j + 1) * tile_cols],
                        ).then_inc(a_sem, 16)

            @block.vector
            def _(vector):
                # chase the two DMA streams in arrival order
                order = sorted(range(n_tiles), key=lambda j: (owner[j][1], j))
                for j in order:
                    sem = s_sem if owner[j][0] == "s" else a_sem
                    vector.wait_ge(sem, owner[j][1])
                    if g == 1:
                        src = tiles[j][:]
                    else:
                        src = tiles[j][:].rearrange("p (g d) -> p g d", g=g)
                    vector.tensor_reduce(
                        out=ot[:, j * g:(j + 1) * g],
                        in_=src,
                        axis=mybir.AxisListType.X,
                        op=mybir.AluOpType.add,
                    ).then_inc(v_sem, 1)

    return nc, blk


def _build_nc(n_dma=NBLK):
    """One core's program: DMA [128, 2304] f32 in `n_dma` column tiles,
    VectorE-reduce each tile over its free dim in BLK-sized chunks,
    DMA the [128, NBLK] block sums out."""
    nc = bacc.Bacc(None, target_bir_lowering=False)
    w = nc.declare_dram_parameter("w", [P, COLS], mybir.dt.float32, isOutput=False)
    out = nc.declare_dram_parameter("out", [P, NBLK], mybir.dt.float32, isOutput=True)

    tile_cols = COLS // n_dma
    blk_per_tile = tile_cols // BLK

    with tile.TileContext(nc) as tc:
        with (
            tc.tile_pool(name="wpool", bufs=min(3, n_dma)) as wp,
            tc.tile_pool(name="opool", bufs=1) as op,
        ):
            ot = op.tile([P, NBLK], mybir.dt.float32)
            for j in range(n_dma):
                t = wp.tile([P, tile_cols], mybir.dt.float32)
                nc.sync.dma_start(out=t[:], in_=w[:, j * tile_cols:(j + 1) * tile_cols])
                if blk_per_tile == 1:
                    nc.vector.tensor_reduce(
                        out=ot[:, j:j + 1], in_=t[:],
                        axis=mybir.AxisListType.X, op=mybir.AluOpType.add,
                    )
                else:
                    nc.vector.tensor_reduce(
                        out=ot[:, j * blk_per_tile:(j + 1) * blk_per_tile],
                        in_=t[:].rearrange("p (g d) -> p g d", g=blk_per_tile),
                        axis=mybir.AxisListType.X, op=mybir.AluOpType.add,
                    )
            nc.sync.dma_start(out=out[:], in_=ot[:])
    nc.compile()
    return nc, BLK


def _build_nc_fast():
    """Stripped raw bass: no entry barrier / const memsets / Block exit
    barrier.  Host packs each core's 294,912 f32 as [576, 512] so every
    DMA row is exactly 2048 B (one clean DGE packet).  5 input tiles
    ([128,512] x4 + [64,512]); Sync and Scalar HWDGE queues stream in
    parallel; VectorE reduces each tile to per-partition sums as it
    lands; Sync DMAs the [128,5] block-sum tile out and waits for its
    completion (no trailing drain needed)."""
    nc = bass.Bass(target_bir_lowering=False)

    # drop the constructor's const memsets and all-engine barrier; our
    # explicit semaphore protocol doesn't need them (NRT zeroes sems at
    # load) and they cost ~2us of serial entry time
    bb = nc.main_func.blocks[0]
    drop = ("InstMemset", "InstDrain", "InstEventSemaphore")
    bb.instructions[:] = [
        i for i in bb.instructions if type(i).__name__ not in drop
    ]

    w = nc.declare_dram_parameter("w", [576, 512], mybir.dt.float32, isOutput=False)
    out = nc.declare_dram_parameter("out", [P, 5], mybir.dt.float32, isOutput=True)

    with ExitStack() as ctx:
        s_sem = ctx.enter_context(nc.semaphore("s_sem"))
        a_sem = ctx.enter_context(nc.semaphore("a_sem"))
        v_sem = ctx.enter_context(nc.semaphore("v_sem"))
        tiles = [
            ctx.enter_context(
                nc.sbuf_tensor(f"t{j}", [128 if j < 4 else 64, 512],
                               mybir.dt.float32)
            )
            for j in range(5)
        ]
        ot = ctx.enter_context(nc.sbuf_tensor("ot", [P, 5], mybir.dt.float32))

        # sync streams tiles 0,2; scalar streams 1,3,4 (4 is half-size)
        nc.sync.dma_start(out=tiles[0][:], in_=w[0:128, :]).then_inc(s_sem, 16)
        nc.sync.dma_start(out=tiles[2][:], in_=w[256:384, :]).then_inc(s_sem, 16)
        nc.scalar.dma_start(out=tiles[1][:], in_=w[128:256, :]).then_inc(a_sem, 16)
        nc.scalar.dma_start(out=tiles[3][:], in_=w[384:512, :]).then_inc(a_sem, 16)
        nc.scalar.dma_start(out=tiles[4][:], in_=w[512:576, :]).then_inc(a_sem, 16)

        # vector chases both queues in expected arrival order
        chase = [(s_sem, 16, 0), (a_sem, 16, 1), (s_sem, 32, 2),
                 (a_sem, 32, 3), (a_sem, 48, 4)]
        for sem, thresh, j in chase:
            nc.vector.wait_ge(sem, thresh)
            rows = 128 if j < 4 else 64
            nc.vector.tensor_reduce(
                out=ot[0:rows, j:j + 1], in_=tiles[j][:],
                axis=mybir.AxisListType.X, op=mybir.AluOpType.add,
            ).then_inc(v_sem, 1)

        nc.sync.wait_ge(v_sem, 5)
        nc.sync.dma_start(out=out[:], in_=ot[:]).then_inc(s_sem, 16)
        nc.sync.wait_ge(s_sem, 48)

    return nc, 512


def _get_nc(variant="fast"):
    if variant not in _NC_CACHE:
        if variant == "tile":
            _NC_CACHE[variant] = _build_nc()
        elif variant == "fast":
            _NC_CACHE[variant] = _build_nc_fast()
        else:
            _NC_CACHE[variant] = _build_nc_raw(n_tiles=int(variant[3:]))
    return _NC_CACHE[variant]


def _run_device(wl_flat, variant="fast", trace=False):
    """wl_flat: contiguous f32 [D*F]. Returns (w_sum [D] f64, results obj)."""
    nc, blk = _get_nc(variant)
    if variant == "fast":
        in_maps = [
            {"w": np.ascontiguousarray(
                wl_flat[c * ELEMS_PER_CORE:(c + 1) * ELEMS_PER_CORE]
                .reshape(576, 512))}
            for c in range(N_CORES)
        ]
    else:
        in_maps = [
            {"w": np.ascontiguousarray(
                wl_flat[c * ELEMS_PER_CORE:(c + 1) * ELEMS_PER_CORE]
                .reshape(P, COLS))}
            for c in range(N_CORES)
        ]
    res = run_bass_kernel_spmd(
        nc, in_maps, core_ids=list(range(N_CORES)), trace=trace
    )
    if variant == "fast":
        per_core = []
        for r in res.results:
            o = np.asarray(r["out"], dtype=np.float64)       # [128, 5]
            per_core.append(np.concatenate([o[:, 0], o[:, 1], o[:, 2],
                                            o[:, 3], o[:64, 4]]))
        blocks = np.concatenate(per_core)                    # 8 * 576 block sums
    else:
        blocks = np.concatenate(
            [np.asarray(r["out"], dtype=np.float64).reshape(-1)
             for r in res.results]
        )                               # sums of blk consecutive flat elems
    w_sum = blocks.reshape(D, F // blk).sum(axis=1)          # [768]
    return w_sum, res


def kernel(ffn_input, W, b, target_layer, target_token_positions):
    tl = int(target_layer)
    wl_flat = np.ascontiguousarray(W[tl], dtype=np.float32).reshape(-1)
    w_sum, _ = _run_device(wl_flat, variant=VARIANT)

    pos = np.asarray(target_token_positions).astype(np.int64)
    valid = (pos >= 0) & (pos < S)
    safe = np.clip(pos, 0, S - 1)
    x = np.asarray(ffn_input)[np.arange(B), safe].astype(np.float64)   # [16, 768]
    row = x @ w_sum / F + float(np.asarray(b[tl], dtype=np.float64).mean())
    return np.where(valid, row, 0.0).astype(np.float32)


# revision 14
# speedup vs baseline: 1.5107x; 1.0251x over previous
"""Bass/Trainium2 kernel for nn_GPT2FFNInputModel (segment_reduce, memory regime).

Reference computes, for B=16 gathered token rows x[b] = ffn_input[b, pos[b]]:
    out[b] = mean_f( x[b] @ W[tl] + b[tl] )        (masked to 0 for invalid pos)

The mean over F folds through the matmul:
    out[b] = (x[b] . w_sum) / F + mean(b[tl]),   w_sum[d] = sum_f W[tl][d, f]

so the only bulk memory work is the row-sum (segment reduce) of W[tl]
(768 x 3072 f32 = 9.4 MB).  That reduction runs on 8 NeuronCores, each
reducing a contiguous 1/8th of W[tl] laid out as [128 partitions x 2304],
producing 768-element block partial sums.  The tiny [16,768] gather, the
16x768 dot, bias mean and validity mask run on host (48 KB of data).
"""

from contextlib import ExitStack

import numpy as np

import concourse.bass as bass
import concourse.mybir as mybir
import concourse.tile as tile
from concourse import bacc
from concourse.bass_utils import run_bass_kernel_spmd

B, S, D, F = 16, 2048, 768, 3072
N_CORES = 8
P = 128
ELEMS_PER_CORE = D * F // N_CORES      # 294912 contiguous f32 per core
COLS = ELEMS_PER_CORE // P             # 2304 per partition
BLK = 768                              # reduction block; F % BLK == 0 keeps
NBLK = COLS // BLK                     # 3   row boundaries block-aligned

VARIANT = "fast"                       # which device program kernel() uses

_NC_CACHE = {}


def _build_nc_raw(n_tiles=4):
    """Raw bass (no TileContext): explicit semaphores, minimal engine set.
    Sync and Scalar (both HWDGE) each issue half the input DMAs in
    parallel; VectorE reduces each tile as it lands; Sync DMAs the block
    sums out.  Avoids Tile's multi-microsecond entry/exit barriers."""
    tile_cols = COLS // n_tiles                  # per-tile free dim
    blk = 768
    while tile_cols % blk:                       # largest BLK dividing both
        blk //= 2                                # tile_cols and F
    g = tile_cols // blk
    nblk_total = COLS // blk

    nc = bass.Bass(target_bir_lowering=False)
    w = nc.declare_dram_parameter("w", [P, COLS], mybir.dt.float32, isOutput=False)
    out = nc.declare_dram_parameter(
        "out", [P, nblk_total], mybir.dt.float32, isOutput=True
    )

    with ExitStack() as ctx:
        s_sem = ctx.enter_context(nc.semaphore("s_sem"))
        a_sem = ctx.enter_context(nc.semaphore("a_sem"))
        v_sem = ctx.enter_context(nc.semaphore("v_sem"))
        tiles = [
            ctx.enter_context(
                nc.sbuf_tensor(f"t{j}", [P, tile_cols], mybir.dt.float32)
            )
            for j in range(n_tiles)
        ]
        ot = ctx.enter_context(
            nc.sbuf_tensor("ot", [P, nblk_total], mybir.dt.float32)
        )

        # tile j -> (engine, completion threshold on that engine's sem)
        half = (n_tiles + 1) // 2
        owner = [("s", 16 * (j + 1)) if j < half else ("a", 16 * (j - half + 1))
                 for j in range(n_tiles)]

        with nc.Block() as block:

            @block.sync
            def _(sync):
                for j in range(n_tiles):
                    if owner[j][0] == "s":
                        sync.dma_start(
                            out=tiles[j][:],
                            in_=w[:, j * tile_cols:(j + 1) * tile_cols],
                        ).then_inc(s_sem, 16)
                sync.wait_ge(v_sem, n_tiles)
                sync.dma_start(out=out[:], in_=ot[:]).then_inc(s_sem, 16)
                sync.wait_ge(s_sem, 16 * (half + 1))

            @block.scalar
            def _(scalar):
                for j in range(n_tiles):
                    if owner[j][0] == "a":
                        scalar.dma_start(
                            out=tiles[j][:],
                            in_=w[:, j * tile_cols:(j + 1) * tile_cols],
                        ).then_inc(a_sem, 16)

            @block.vector
            def _(vector):
                # chase the two DMA streams in arrival order
                order = sorted(range(n_tiles), key=lambda j: (owner[j][1], j))
                for j in order:
                    sem = s_sem if owner[j][0] == "s" else a_sem
                    vector.wait_ge(sem, owner[j][1])
                    if g == 1:
                        src = tiles[j][:]
                    else:
                        src = tiles[j][:].rearrange("p (g d) -> p g d", g=g)
                    vector.tensor_reduce(
                        out=ot[:, j * g:(j + 1) * g],
                        in_=src,
                        axis=mybir.AxisListType.X,
                        op=mybir.AluOpType.add,
                    ).then_inc(v_sem, 1)

    return nc, blk


def _build_nc(n_dma=NBLK):
    """One core's program: DMA [128, 2304] f32 in `n_dma` column tiles,
    VectorE-reduce each tile over its free dim in BLK-sized chunks,
    DMA the [128, NBLK] block sums out."""
    nc = bacc.Bacc(None, target_bir_lowering=False)
    w = nc.declare_dram_parameter("w", [P, COLS], mybir.dt.float32, isOutput=False)
    out = nc.declare_dram_parameter("out", [P, NBLK], mybir.dt.float32, isOutput=True)

    tile_cols = COLS // n_dma
    blk_per_tile = tile_cols // BLK

    with tile.TileContext(nc) as tc:
        with (
            tc.tile_pool(name="wpool", bufs=min(3, n_dma)) as wp,
            tc.tile_pool(name="opool", bufs=1) as op,
        ):
            ot = op.tile([P, NBLK], mybir.dt.float32)
            for j in range(n_dma):
                t = wp.tile([P, tile_cols], mybir.dt.float32)
                nc.sync.dma_start(out=t[:], in_=w[:, j * tile_cols:(j + 1) * tile_cols])
                if blk_per_tile == 1:
                    nc.vector.tensor_reduce(
                        out=ot[:, j:j + 1], in_=t[:],
                        axis=mybir.AxisListType.X, op=mybir.AluOpType.add,
                    )
                else:
                    nc.vector.tensor_reduce(
                        out=ot[:, j * blk_per_tile:(j + 1) * blk_per_tile],
                        in_=t[:].rearrange("p (g d) -> p g d", g=blk_per_tile),
                        axis=mybir.AxisListType.X, op=mybir.AluOpType.add,
                    )
            nc.sync.dma_start(out=out[:], in_=ot[:])
    nc.compile()
    return nc, BLK


def _build_nc_fast():
    """Stripped raw bass: no entry barrier / const memsets / Block exit
    barrier.  Host packs each core's 294,912 f32 as [576, 512] so every
    DMA row is exactly 2048 B (one clean DGE packet).  5 input tiles
    ([128,512] x4 + [64,512]); Sync and Scalar HWDGE queues stream in
    parallel; VectorE reduces each tile to per-partition sums as it
    lands; Sync DMAs the [128,5] block-sum tile out and waits for its
    completion (no trailing drain needed)."""
    nc = bass.Bass(target_bir_lowering=False)

    # drop the constructor's const memsets and all-engine barrier; our
    # explicit semaphore protocol doesn't need them (NRT zeroes sems at
    # load) and they cost ~2us of serial entry time
    bb = nc.main_func.blocks[0]
    drop = ("InstMemset", "InstDrain", "InstEventSemaphore")
    bb.instructions[:] = [
        i for i in bb.instructions if type(i).__name__ not in drop
    ]

    w = nc.declare_dram_parameter("w", [576, 512], mybir.dt.float32, isOutput=False)
    out = nc.declare_dram_parameter("out", [P, 5], mybir.dt.float32, isOutput=True)

    with ExitStack() as ctx:
        s_sem = ctx.enter_context(nc.semaphore("s_sem"))
        a_sem = ctx.enter_context(nc.semaphore("a_sem"))
        v_sem = ctx.enter_context(nc.semaphore("v_sem"))
        tiles = [
            ctx.enter_context(
                nc.sbuf_tensor(f"t{j}", [128 if j < 4 else 64, 512],
                               mybir.dt.float32)
            )
            for j in range(5)
        ]
        ot = ctx.enter_context(nc.sbuf_tensor("ot", [P, 5], mybir.dt.float32))

        # sync streams tiles 0,2; scalar streams 1,3,4 (4 is half-size)
        nc.sync.dma_start(out=tiles[0][:], in_=w[0:128, :]).then_inc(s_sem, 16)
        nc.sync.dma_start(out=tiles[2][:], in_=w[256:384, :]).then_inc(s_sem, 16)
        nc.scalar.dma_start(out=tiles[1][:], in_=w[128:256, :]).then_inc(a_sem, 16)
        nc.scalar.dma_start(out=tiles[3][:], in_=w[384:512, :]).then_inc(a_sem, 16)
        nc.scalar.dma_start(out=tiles[4][:], in_=w[512:576, :]).then_inc(a_sem, 16)

        # vector chases both queues in expected arrival order
        chase = [(s_sem, 16, 0), (a_sem, 16, 1), (s_sem, 32, 2),
                 (a_sem, 32, 3), (a_sem, 48, 4)]
        for sem, thresh, j in chase:
            nc.vector.wait_ge(sem, thresh)
            rows = 128 if j < 4 else 64
            nc.vector.tensor_reduce(
                out=ot[0:rows, j:j + 1], in_=tiles[j][:],
                axis=mybir.AxisListType.X, op=mybir.AluOpType.add,
            ).then_inc(v_sem, 1)

        nc.sync.wait_ge(v_sem, 5)
        nc.sync.dma_start(out=out[:], in_=ot[:]).then_inc(s_sem, 16)
        nc.sync.wait_ge(s_sem, 48)

    return nc, 512


def _build_nc_f2():
    """fast + stripped regmoves, DGE warm-up DMAs, all-128-partition tiles
    with a small last tile to shrink the post-stream tail.

    Flat per-core layout [294912] viewed as [576, 512]:
      t0 [128,512] @0        sync     t1 [128,512] @65536   scalar
      t2 [128,512] @131072   sync     t3 [128,512] @196608  scalar
      t4 [128,256] @262144   scalar (last, half-width)
    Each tile row is one reduce block (512 or 256 consecutive flat f32)."""
    nc = bass.Bass(target_bir_lowering=False)
    bb = nc.main_func.blocks[0]
    drop = ("InstMemset", "InstDrain", "InstEventSemaphore", "InstRegisterMove")
    bb.instructions[:] = [
        i for i in bb.instructions if type(i).__name__ not in drop
    ]

    w = nc.declare_dram_parameter("w", [576, 512], mybir.dt.float32, isOutput=False)
    out = nc.declare_dram_parameter("out", [P, 5], mybir.dt.float32, isOutput=True)

    def ap(off, parts, cols, stride):
        return bass.AP(w, off, [[stride, parts], [1, cols]])

    with ExitStack() as ctx:
        s_sem = ctx.enter_context(nc.semaphore("s_sem"))
        a_sem = ctx.enter_context(nc.semaphore("a_sem"))
        v_sem = ctx.enter_context(nc.semaphore("v_sem"))
        tiles = [
            ctx.enter_context(
                nc.sbuf_tensor(f"t{j}", [128, 512 if j < 4 else 256],
                               mybir.dt.float32)
            )
            for j in range(5)
        ]
        warm = ctx.enter_context(nc.sbuf_tensor("warm", [1, 1], mybir.dt.float32))
        ot = ctx.enter_context(nc.sbuf_tensor("ot", [P, 5], mybir.dt.float32))

        # 4B warm-ups absorb each HWDGE queue's wake-up latency
        nc.sync.dma_start(out=warm[:], in_=ap(0, 1, 1, 1)).then_inc(s_sem, 16)
        nc.scalar.dma_start(out=warm[:], in_=ap(0, 1, 1, 1)).then_inc(a_sem, 16)

        nc.sync.dma_start(out=tiles[0][:], in_=ap(0, 128, 512, 512)).then_inc(s_sem, 16)
        nc.sync.dma_start(out=tiles[2][:], in_=ap(131072, 128, 512, 512)).then_inc(s_sem, 16)
        nc.scalar.dma_start(out=tiles[1][:], in_=ap(65536, 128, 512, 512)).then_inc(a_sem, 16)
        nc.scalar.dma_start(out=tiles[3][:], in_=ap(196608, 128, 512, 512)).then_inc(a_sem, 16)
        nc.scalar.dma_start(out=tiles[4][:], in_=ap(262144, 128, 256, 256)).then_inc(a_sem, 16)

        chase = [(s_sem, 32, 0), (a_sem, 32, 1), (s_sem, 48, 2),
                 (a_sem, 48, 3), (a_sem, 64, 4)]
        for sem, thresh, j in chase:
            nc.vector.wait_ge(sem, thresh)
            nc.vector.tensor_reduce(
                out=ot[:, j:j + 1], in_=tiles[j][:],
                axis=mybir.AxisListType.X, op=mybir.AluOpType.add,
            ).then_inc(v_sem, 1)

        nc.sync.wait_ge(v_sem, 5)
        nc.sync.dma_start(out=out[:], in_=ot[:]).then_inc(s_sem, 16)
        nc.sync.wait_ge(s_sem, 64)

    return nc, None


def _get_nc(variant="fast"):
    if variant not in _NC_CACHE:
        if variant == "tile":
            _NC_CACHE[variant] = _build_nc()
        elif variant == "fast":
            _NC_CACHE[variant] = _build_nc_fast()
        elif variant == "f2":
            _NC_CACHE[variant] = _build_nc_f2()
        else:
            _NC_CACHE[variant] = _build_nc_raw(n_tiles=int(variant[3:]))
    return _NC_CACHE[variant]


def _run_device(wl_flat, variant="fast", trace=False):
    """wl_flat: contiguous f32 [D*F]. Returns (w_sum [D] f64, results obj)."""
    nc, blk = _get_nc(variant)
    if variant in ("fast", "f2"):
        in_maps = [
            {"w": np.ascontiguousarray(
                wl_flat[c * ELEMS_PER_CORE:(c + 1) * ELEMS_PER_CORE]
                .reshape(576, 512))}
            for c in range(N_CORES)
        ]
    else:
        in_maps = [
            {"w": np.ascontiguousarray(
                wl_flat[c * ELEMS_PER_CORE:(c + 1) * ELEMS_PER_CORE]
                .reshape(P, COLS))}
            for c in range(N_CORES)
        ]
    res = run_bass_kernel_spmd(
        nc, in_maps, core_ids=list(range(N_CORES)), trace=trace
    )
    if variant == "f2":
        # mixed 512/256-elem blocks; map each block to its W-row by offset
        offs, vals = [], []
        p = np.arange(128)
        for c, r in enumerate(res.results):
            o = np.asarray(r["out"], dtype=np.float64)       # [128, 5]
            base = c * ELEMS_PER_CORE
            for j in range(4):
                offs.append(base + j * 65536 + p * 512)
                vals.append(o[:, j])
            offs.append(base + 262144 + p * 256)
            vals.append(o[:, 4])
        rows = np.concatenate(offs) // F
        w_sum = np.bincount(rows, weights=np.concatenate(vals), minlength=D)
        return w_sum, res
    if variant == "fast":
        per_core = []
        for r in res.results:
            o = np.asarray(r["out"], dtype=np.float64)       # [128, 5]
            per_core.append(np.concatenate([o[:, 0], o[:, 1], o[:, 2],
                                            o[:, 3], o[:64, 4]]))
        blocks = np.concatenate(per_core)                    # 8 * 576 block sums
    else:
        blocks = np.concatenate(
            [np.asarray(r["out"], dtype=np.float64).reshape(-1)
             for r in res.results]
        )                               # sums of blk consecutive flat elems
    w_sum = blocks.reshape(D, F // blk).sum(axis=1)          # [768]
    return w_sum, res


def kernel(ffn_input, W, b, target_layer, target_token_positions):
    tl = int(target_layer)
    wl_flat = np.ascontiguousarray(W[tl], dtype=np.float32).reshape(-1)
    w_sum, _ = _run_device(wl_flat, variant=VARIANT)

    pos = np.asarray(target_token_positions).astype(np.int64)
    valid = (pos >= 0) & (pos < S)
    safe = np.clip(pos, 0, S - 1)
    x = np.asarray(ffn_input)[np.arange(B), safe].astype(np.float64)   # [16, 768]
    row = x @ w_sum / F + float(np.asarray(b[tl], dtype=np.float64).mean())
    return np.where(valid, row, 0.0).astype(np.float32)
